# revision 4
# baseline (speedup 1.0000x reference)
"""SMPL (shape blend + pose blend + LBS skinning) Bass kernel for 8 TRN2 NeuronCores.

Data-parallel over batch: B=1024 -> 128 per core. All SMPL buffers replicated.

v2: bf16 main loop (fp32 matmuls are 4 cyc/row on TensorE; bf16 is 1),
host-precomputed joint regressor (J = [beta|1] @ JS2), ScalarE handles
PSUM->SBUF copies, DVE combine runs in bf16 2x mode.

Per-core pipeline:
  pose -> Rodrigues (fp32, ScalarE Sin half-angle + DVE) -> R [128,24*9]
  beta,lrotmin -> coeff [128,218] -> TensorE transpose -> coeffT (bf16 lhsT)
  J = [beta|1] @ JS2 (host-precomputed [11,72] regressor contraction)
  FK over the kinematic tree: 9 level-groups of broadcast-strided DVE ops
  G -> 12 TensorE transposes -> gat [24, 12*128] bf16 (lhsT of skinning matmul)
  main loop over vertex chunks (ch=512):
      vp   = coeffT.T @ dirs_chunk            (TensorE bf16, K=218)
      T_e  = gat_e.T @ wt_chunk               (TensorE bf16, K=24, 12 planes)
      out_m = sum_c T_mc*vp_c + T_m3          (DVE bf16 2x; ScalarE copies)
Output per core: [128, 3, 6890] bf16 plane-major; host reassembles [1024, 6890, 3].
"""

import sys
import numpy as np
import ml_dtypes

for _p in ("/opt/trn_rl_repo",):
    if _p not in sys.path:
        sys.path.append(_p)

import concourse.bass as bass
import concourse.tile as tile
import concourse.mybir as mybir
from concourse import bacc
from concourse.bass_utils import run_bass_kernel_spmd
from concourse.alu_op_type import AluOpType

F32 = mybir.dt.float32
BF16 = mybir.dt.bfloat16
NP_BF16 = ml_dtypes.bfloat16

N_CORES = 8
B = 1024
B_LOC = B // N_CORES  # 128
NV = 6890
NJ = 24
NPD = 207         # pose blend coeffs
KC = 218          # 10 beta + 207 lrotmin + 1 const
CH = 512          # vertex chunk

# FK level groups: (child_start, n_children, parent_start, parent_broadcast)
FK_GROUPS = [
    (1, 3, 0, True),
    (4, 3, 1, False),
    (7, 3, 4, False),
    (10, 3, 7, False),
    (13, 2, 9, True),
    (15, 3, 12, False),
    (18, 2, 16, False),
    (20, 2, 18, False),
    (22, 2, 20, False),
]

CFG = {
    "trace": False,
    "debug": False,
}

_CACHE = {}


def build_program(cfg):
    key = ("bf16", CH)
    if key in _CACHE:
        return _CACHE[key]

    nc = bacc.Bacc("TRN2", target_bir_lowering=False, debug=False)

    # ---- DRAM parameters ----
    pose_d = nc.dram_tensor("pose", [B_LOC, 72], F32, kind="ExternalInput")
    beta_d = nc.dram_tensor("beta", [B_LOC, 10], F32, kind="ExternalInput")
    dirs_d = nc.dram_tensor("dirs", [KC, 3, NV], BF16, kind="ExternalInput")
    wt_d = nc.dram_tensor("wt", [NJ, NV], BF16, kind="ExternalInput")
    js2_d = nc.dram_tensor("js2", [11, 72], F32, kind="ExternalInput")
    ident_d = nc.dram_tensor("ident", [128, 128], F32, kind="ExternalInput")
    out_d = nc.dram_tensor("out", [B_LOC, 3, NV], BF16, kind="ExternalOutput")

    with tile.TileContext(nc) as tc:
        with (
            tc.tile_pool(name="const", bufs=1) as constp,
            tc.tile_pool(name="state", bufs=1) as statep,
            tc.tile_pool(name="scr", bufs=1) as scrp,
        ):
            # ---- const loads ----
            ident = constp.tile([128, 128], F32)
            nc.sync.dma_start(ident[:, :], ident_d.ap())
            wt_sb = constp.tile([NJ, NV], BF16)
            nc.sync.dma_start(wt_sb[:, :], wt_d.ap())
            js2_sb = constp.tile([11, 72], F32)
            nc.sync.dma_start(js2_sb[:, :], js2_d.ap())
            pose_sb = statep.tile([B_LOC, 72], F32)
            nc.sync.dma_start(pose_sb[:, :], pose_d.ap())

            # ---- Rodrigues (fp32) ----
            V = nc.vector
            S = nc.scalar
            sq = scrp.tile([B_LOC, 72], F32, tag="sq")
            V.tensor_mul(sq[:, :], pose_sb[:, :], pose_sb[:, :])
            sq3 = sq[:, :].rearrange("p (j c) -> p c j", c=3)
            th2 = scrp.tile([B_LOC, NJ], F32, tag="th2")
            V.tensor_add(th2[:, :], sq3[:, 0, :], sq3[:, 1, :])
            V.tensor_add(th2[:, :], th2[:, :], sq3[:, 2, :])
            cbias = constp.tile([128, 2], F32)
            V.memset(cbias[:, 0:1], 1e-8)
            V.memset(cbias[:, 1:2], float(np.pi / 2))
            theta = scrp.tile([B_LOC, NJ], F32, tag="theta")
            S.activation(theta[:, :], th2[:, :], mybir.ActivationFunctionType.Sqrt,
                         bias=cbias[0:B_LOC, 0:1])
            invt = scrp.tile([B_LOC, NJ], F32, tag="invt")
            V.reciprocal(invt[:, :], theta[:, :])
            sh = scrp.tile([B_LOC, NJ], F32, tag="sh")
            S.activation(sh[:, :], theta[:, :], mybir.ActivationFunctionType.Sin, scale=0.5)
            chh = scrp.tile([B_LOC, NJ], F32, tag="chh")
            S.activation(chh[:, :], theta[:, :], mybir.ActivationFunctionType.Sin,
                         scale=0.5, bias=cbias[0:B_LOC, 1:2])
            s_t = scrp.tile([B_LOC, NJ], F32, tag="s_t")
            V.scalar_tensor_tensor(s_t[:, :], sh[:, :], 2.0, chh[:, :], AluOpType.mult, AluOpType.mult)
            shsq = scrp.tile([B_LOC, NJ], F32, tag="shsq")
            V.tensor_mul(shsq[:, :], sh[:, :], sh[:, :])
            c_t = scrp.tile([B_LOC, NJ], F32, tag="c_t")
            V.tensor_scalar(c_t[:, :], shsq[:, :], -2.0, 1.0, AluOpType.mult, AluOpType.add)
            omc = scrp.tile([B_LOC, NJ], F32, tag="omc")
            V.tensor_scalar_mul(omc[:, :], shsq[:, :], 2.0)
            ax = scrp.tile([B_LOC, 72], F32, tag="ax")
            ax3 = ax[:, :].rearrange("p (j c) -> p c j", c=3)
            p3 = pose_sb[:, :].rearrange("p (j c) -> p c j", c=3)
            for ci in range(3):
                V.tensor_mul(ax3[:, ci, :], p3[:, ci, :], invt[:, :])
            prods = {}
            for name, (a, b_) in {
                "xx": (0, 0), "yy": (1, 1), "zz": (2, 2),
                "xy": (0, 1), "xz": (0, 2), "yz": (1, 2),
            }.items():
                t = scrp.tile([B_LOC, NJ], F32, tag="prod_" + name)
                V.tensor_mul(t[:, :], ax3[:, a, :], ax3[:, b_, :])
                V.tensor_mul(t[:, :], t[:, :], omc[:, :])
                prods[name] = t
            for name, a in {"sx": 0, "sy": 1, "sz": 2}.items():
                t = scrp.tile([B_LOC, NJ], F32, tag="prod_" + name)
                V.tensor_mul(t[:, :], s_t[:, :], ax3[:, a, :])
                prods[name] = t
            r9 = statep.tile([B_LOC, NJ * 9], F32)
            r9e = r9[:, :].rearrange("p (j e) -> p e j", e=9)
            ENTRIES = [
                ("add", "c", "xx"), ("sub", "xy", "sz"), ("add", "xz", "sy"),
                ("add", "xy", "sz"), ("add", "c", "yy"), ("sub", "yz", "sx"),
                ("sub", "xz", "sy"), ("add", "yz", "sx"), ("add", "c", "zz"),
            ]
            for e, (op, a, b_) in enumerate(ENTRIES):
                ta = c_t if a == "c" else prods[a]
                fn = V.tensor_add if op == "add" else V.tensor_sub
                fn(r9e[:, e, :], ta[:, :], prods[b_][:, :])

            # ---- coeff & transposes ----
            coeff = statep.tile([B_LOC, KC], F32)
            nc.sync.dma_start(coeff[:, 0:10], beta_d.ap())
            V.tensor_copy(coeff[:, 10:217], r9[:, 9:216])
            lr9 = coeff[:, 10:217].rearrange("p (j e) -> p e j", e=9)
            for e in (0, 4, 8):
                V.tensor_scalar_add(lr9[:, e, :], lr9[:, e, :], -1.0)
            V.memset(coeff[:, 217:218], 1.0)

            with tc.tile_pool(name="psA", bufs=2, space="PSUM") as psA:
                pt1 = psA.tile([128, 128], F32, tag="tp")
                nc.tensor.transpose(pt1[:, :], coeff[:, 0:128], ident[:, :])
                coeffT_a = statep.tile([128, B_LOC], BF16)
                V.tensor_copy(coeffT_a[:, :], pt1[:, :])
                # fp32 betaT (rows 0..9 of pt1) + ones row for the J matmul
                betaT1 = statep.tile([11, B_LOC], F32)
                V.memset(betaT1[:, :], 1.0)
                V.tensor_copy(betaT1[0:10, :], pt1[0:10, :])
                pt2 = psA.tile([128, 128], F32, tag="tp")
                nc.tensor.transpose(pt2[0:90, :], coeff[:, 128:218], ident[:, :])
                coeffT_b = statep.tile([90, B_LOC], BF16)
                V.tensor_copy(coeffT_b[:, :], pt2[0:90, :])

                # ---- J = [beta | 1] @ JS2 (host-precomputed regressor) ----
                pj = psA.tile([B_LOC, 72], F32, tag="pj")
                nc.tensor.matmul(pj[:, :], betaT1[:, :], js2_sb[:, :],
                                 start=True, stop=True)
                j_sb = statep.tile([B_LOC, 72], F32)
                V.tensor_copy(j_sb[:, :], pj[:, :])

            # ---- J_rel ----
            jrel = statep.tile([B_LOC, 72], F32)
            jv = j_sb[:, :].rearrange("p (j c) -> p j c", c=3)
            jrv = jrel[:, :].rearrange("p (j c) -> p j c", c=3)
            V.tensor_copy(jrel[:, 0:3], j_sb[:, 0:3])
            V.tensor_sub(jrv[:, 1:4], jv[:, 1:4], jv[:, 0:1].broadcast_to([B_LOC, 3, 3]))
            V.tensor_sub(jrv[:, 4:12], jv[:, 4:12], jv[:, 1:9])
            V.tensor_sub(jrv[:, 12:15], jv[:, 12:15], jv[:, 9:10].broadcast_to([B_LOC, 3, 3]))
            V.tensor_sub(jrv[:, 15:18], jv[:, 15:18], jv[:, 12:15])
            V.tensor_sub(jrv[:, 18:24], jv[:, 18:24], jv[:, 16:22])

            # ---- local transforms Gl [128, 24*12] (3x4 row-major [R|t]) ----
            gl = statep.tile([B_LOC, NJ * 12], F32)
            gl4 = gl[:, :].rearrange("p (j m n) -> p j m n", m=3, n=4)
            r94 = r9[:, :].rearrange("p (j m n) -> p j m n", m=3, n=3)
            V.tensor_copy(gl4[:, :, :, 0:3], r94[:, :, :, :])
            V.tensor_copy(gl4[:, :, :, 3:4], jrv[:, :, :].unsqueeze(3))

            # ---- forward kinematics ----
            gw = statep.tile([B_LOC, NJ * 12], F32)
            gw4 = gw[:, :].rearrange("p (j m n) -> p j m n", m=3, n=4)
            V.tensor_copy(gw[:, 0:12], gl[:, 0:12])
            fktmp = scrp.tile([B_LOC, 3 * 12], F32, tag="fktmp")
            for (c0, ncld, p0, bc) in FK_GROUPS:
                child = gw4[:, c0:c0 + ncld]
                loc = gl4[:, c0:c0 + ncld]
                par = gw4[:, p0:p0 + (1 if bc else ncld)]
                tmpv = fktmp[:, 0:ncld * 12].rearrange("p (j m n) -> p j m n", m=3, n=4)
                shp = [B_LOC, ncld, 3, 4]
                for k in range(3):
                    in0 = loc[:, :, k:k + 1, :].broadcast_to(shp)
                    pk = par[:, 0:1, :, k:k + 1] if bc else par[:, :, :, k:k + 1]
                    in1 = pk.broadcast_to(shp)
                    if k == 0:
                        V.tensor_mul(child[:, :, :, :], in0, in1)
                    else:
                        V.tensor_mul(tmpv, in0, in1)
                        V.tensor_add(child[:, :, :, :], child[:, :, :, :], tmpv)
                ptr = par[:, 0:1, :, 3:4] if bc else par[:, :, :, 3:4]
                V.tensor_add(child[:, :, :, 3:4], child[:, :, :, 3:4],
                             ptr.broadcast_to([B_LOC, ncld, 3, 1]))

            # ---- rest-pose correction: t_j -= R_j^w @ J_j ----
            ct = scrp.tile([B_LOC, 72], F32, tag="ct")
            ct2 = scrp.tile([B_LOC, 72], F32, tag="ct2")
            ctv = ct[:, :].rearrange("p (j m) -> p j m", m=3).unsqueeze(3)
            ct2v = ct2[:, :].rearrange("p (j m) -> p j m", m=3).unsqueeze(3)
            for k in range(3):
                jk = jv[:, :, k:k + 1].unsqueeze(2).broadcast_to([B_LOC, NJ, 3, 1])
                if k == 0:
                    V.tensor_mul(ctv, gw4[:, :, :, k:k + 1], jk)
                else:
                    V.tensor_mul(ct2v, gw4[:, :, :, k:k + 1], jk)
                    V.tensor_add(ctv, ctv, ct2v)
            V.tensor_sub(gw4[:, :, :, 3:4], gw4[:, :, :, 3:4], ctv)

            # ---- gat via 12 transposes: [24, 12*128] bf16 ----
            gat = statep.tile([NJ, 12 * B_LOC], BF16)
            gwe = gw[:, :].rearrange("p (j e) -> p e j", e=12)
            with tc.tile_pool(name="psT", bufs=3, space="PSUM") as psT:
                for e in range(12):
                    pgt = psT.tile([NJ, B_LOC], F32, tag="gt")
                    nc.tensor.transpose(pgt[:, :], gwe[:, e, :], ident[:, :])
                    V.tensor_copy(gat[:, e * B_LOC:(e + 1) * B_LOC], pgt[:, :])

            # ---- main vertex-chunk loop ----
            _main_loop(nc, tc, coeffT_a, coeffT_b, gat, wt_sb, dirs_d, out_d)

    nc.compile()
    _CACHE[key] = nc
    return nc


def _main_loop(nc, tc, coeffT_a, coeffT_b, gat, wt_sb, dirs_d, out_d):
    V = nc.vector
    S = nc.scalar
    G = nc.gpsimd if CFG.get("gpsimd", True) else nc.vector
    ch = CH
    n_chunks = (NV + ch - 1) // ch
    with (
        tc.tile_pool(name="loop", bufs=3) as loopp,
        tc.tile_pool(name="tsb", bufs=2) as tsbp,
        tc.tile_pool(name="psVP", bufs=1, space="PSUM") as psVP,   # 3 banks
        tc.tile_pool(name="psTT", bufs=2, space="PSUM") as psTT,   # 2x2 banks
    ):
        for ci in range(n_chunks):
            v0 = ci * ch
            sz = min(ch, NV - v0)
            # dirs chunk: [K, 3, sz] bf16, two K-halves
            da = loopp.tile([128, 3, ch], BF16, tag="da")
            nc.sync.dma_start(da[:, :, 0:sz], dirs_d.ap()[0:128, :, v0:v0 + sz])
            db = loopp.tile([90, 3, ch], BF16, tag="db")
            nc.sync.dma_start(db[:, :, 0:sz], dirs_d.ap()[128:KC, :, v0:v0 + sz])

            # vp = coeff @ dirs  (3 c-planes x 2 K-halves)
            pvp = psVP.tile([128, 3 * ch], F32, tag="pvp")
            for c in range(3):
                nc.tensor.matmul(pvp[:, c * ch:c * ch + sz], coeffT_a[:, :],
                                 da[:, c, 0:sz], start=True, stop=False)
                nc.tensor.matmul(pvp[:, c * ch:c * ch + sz], coeffT_b[:, :],
                                 db[:, c, 0:sz], start=False, stop=True)
            # vp copy on DVE
            vp_sb = loopp.tile([B_LOC, 3, ch], BF16, tag="vp")
            if sz == ch:
                V.tensor_copy(vp_sb[:, :, :], pvp[:, :].rearrange("p (c v) -> p c v", v=ch))
            else:
                for c in range(3):
                    V.tensor_copy(vp_sb[:, c, 0:sz], pvp[:, c * ch:c * ch + sz])

            # T planes: 6 groups of 2, rotate psTT; groups 0-4 copied by
            # ScalarE, group 5 by DVE
            t_sb = tsbp.tile([B_LOC, 12, ch], BF16, tag="tsb")
            for g in range(6):
                ptt = psTT.tile([B_LOC, 2 * ch], F32, tag="ptt")
                for h in range(2):
                    e = g * 2 + h
                    nc.tensor.matmul(ptt[:, h * ch:h * ch + sz],
                                     gat[:, e * B_LOC:(e + 1) * B_LOC],
                                     wt_sb[:, v0:v0 + sz], start=True, stop=True)
                eng = S if g < 5 else V
                cp = eng.copy if eng is S else eng.tensor_copy
                if sz == ch:
                    cp(t_sb[:, g * 2:g * 2 + 2, :],
                       ptt[:, :].rearrange("p (h v) -> p h v", v=ch))
                else:
                    for h in range(2):
                        cp(t_sb[:, g * 2 + h, 0:sz], ptt[:, h * ch:h * ch + sz])

            # combine: out_m = sum_c T_{m*4+c} * vp_c + T_{m*4+3}
            # muls: c in {0,1} on DVE, c=2 on GpSimd; adds on GpSimd
            pmul = loopp.tile([B_LOC, 3, 3, ch], BF16, tag="pmul")  # [p, c, m, v]
            for c in range(3):
                eng = V if c < 2 else G
                for m in range(3):
                    eng.tensor_mul(pmul[:, c, m, 0:sz],
                                   t_sb[:, m * 4 + c, 0:sz], vp_sb[:, c, 0:sz])
            q = loopp.tile([B_LOC, 3, ch], BF16, tag="q")
            out_sb = loopp.tile([B_LOC, 3, ch], BF16, tag="outsb")
            t3 = t_sb[:, :, :].rearrange("p (m n) v -> p m n v", n=4)[:, :, 3]
            if sz == ch:
                G.tensor_add(q[:, :, :], pmul[:, 0], pmul[:, 1])
                G.tensor_add(q[:, :, :], q[:, :, :], pmul[:, 2])
                G.tensor_add(out_sb[:, :, :], q[:, :, :], t3)
            else:
                G.tensor_add(q[:, :, 0:sz], pmul[:, 0, :, 0:sz], pmul[:, 1, :, 0:sz])
                G.tensor_add(q[:, :, 0:sz], q[:, :, 0:sz], pmul[:, 2, :, 0:sz])
                G.tensor_add(out_sb[:, :, 0:sz], q[:, :, 0:sz], t3[:, :, 0:sz])
            nc.sync.dma_start(out_d.ap()[:, :, v0:v0 + sz], out_sb[:, :, 0:sz])


def _host_prep(inputs):
    shapedirs = np.asarray(inputs["shapedirs"], np.float32)    # [V,3,10]
    posedirs = np.asarray(inputs["posedirs"], np.float32)      # [V,3,207]
    v_template = np.asarray(inputs["v_template"], np.float32)  # [V,3]
    Jreg = np.asarray(inputs["J_regressor"], np.float32)       # [24,V]
    weights = np.asarray(inputs["weights"], np.float32)        # [V,24]

    dirs = np.empty((KC, 3, NV), np.float32)
    dirs[0:10] = shapedirs.transpose(2, 1, 0)
    dirs[10:217] = posedirs.transpose(2, 1, 0)
    dirs[217] = v_template.T
    # JS2: row s (s<10) = Jreg @ shapedirs[:,:,s] flattened [24*3]; row 10 = Jreg @ template
    js2 = np.empty((11, 72), np.float32)
    js2[0:10] = np.einsum('jv,vcs->sjc', Jreg, shapedirs).reshape(10, 72)
    js2[10] = (Jreg @ v_template).reshape(72)
    rep = {
        "dirs": np.ascontiguousarray(dirs.astype(NP_BF16)),
        "wt": np.ascontiguousarray(weights.T.astype(NP_BF16)),
        "js2": js2,
        "ident": np.eye(128, dtype=np.float32),
    }
    return rep


def kernel(pose, beta, shapedirs, posedirs, v_template, J_regressor, weights):
    cfg = CFG
    nc = build_program(cfg)
    rep = _host_prep(dict(shapedirs=shapedirs, posedirs=posedirs, v_template=v_template,
                          J_regressor=J_regressor, weights=weights))
    pose = np.asarray(pose, np.float32)
    beta = np.asarray(beta, np.float32)
    in_maps = []
    for i in range(N_CORES):
        m = dict(rep)
        m["pose"] = np.ascontiguousarray(pose[i * B_LOC:(i + 1) * B_LOC])
        m["beta"] = np.ascontiguousarray(beta[i * B_LOC:(i + 1) * B_LOC])
        in_maps.append(m)
    res = run_bass_kernel_spmd(nc, in_maps, core_ids=list(range(N_CORES)),
                               trace=cfg.get("trace", False))
    kernel.last_results = res
    out = np.concatenate([np.asarray(res.results[i]["out"]).astype(np.float32)
                          for i in range(N_CORES)], axis=0)
    return np.ascontiguousarray(out.transpose(0, 2, 1))


# revision 7
# speedup vs baseline: 1.1840x; 1.1840x over previous
"""SMPL (shape blend + pose blend + LBS skinning) Bass kernel for 8 TRN2 NeuronCores.

Data-parallel over batch: B=1024 -> 128 per core. All SMPL buffers replicated.

v2: bf16 main loop (fp32 matmuls are 4 cyc/row on TensorE; bf16 is 1),
host-precomputed joint regressor (J = [beta|1] @ JS2), ScalarE handles
PSUM->SBUF copies, DVE combine runs in bf16 2x mode.

Per-core pipeline:
  pose -> Rodrigues (fp32, ScalarE Sin half-angle + DVE) -> R [128,24*9]
  beta,lrotmin -> coeff [128,218] -> TensorE transpose -> coeffT (bf16 lhsT)
  J = [beta|1] @ JS2 (host-precomputed [11,72] regressor contraction)
  FK over the kinematic tree: 9 level-groups of broadcast-strided DVE ops
  G -> 12 TensorE transposes -> gat [24, 12*128] bf16 (lhsT of skinning matmul)
  main loop over vertex chunks (ch=512):
      vp   = coeffT.T @ dirs_chunk            (TensorE bf16, K=218)
      T_e  = gat_e.T @ wt_chunk               (TensorE bf16, K=24, 12 planes)
      out_m = sum_c T_mc*vp_c + T_m3          (DVE bf16 2x; ScalarE copies)
Output per core: [128, 3, 6890] bf16 plane-major; host reassembles [1024, 6890, 3].
"""

import sys
import numpy as np
import ml_dtypes

for _p in ("/opt/trn_rl_repo",):
    if _p not in sys.path:
        sys.path.append(_p)

import concourse.bass as bass
import concourse.tile as tile
import concourse.mybir as mybir
from concourse import bacc
from concourse.bass_utils import run_bass_kernel_spmd
from concourse.alu_op_type import AluOpType

F32 = mybir.dt.float32
BF16 = mybir.dt.bfloat16
NP_BF16 = ml_dtypes.bfloat16

N_CORES = 8
B = 1024
B_LOC = B // N_CORES  # 128
NV = 6890
NJ = 24
NPD = 207         # pose blend coeffs
KC = 218          # 10 beta + 207 lrotmin + 1 const
CH = 512          # vertex chunk

# FK level groups: (child_start, n_children, parent_start, parent_broadcast)
FK_GROUPS = [
    (1, 3, 0, True),
    (4, 3, 1, False),
    (7, 3, 4, False),
    (10, 3, 7, False),
    (13, 2, 9, True),
    (15, 3, 12, False),
    (18, 2, 16, False),
    (20, 2, 18, False),
    (22, 2, 20, False),
]

CFG = {
    "trace": False,
    "debug": False,
}

_CACHE = {}


def build_program(cfg):
    key = ("bf16", CH)
    if key in _CACHE:
        return _CACHE[key]

    nc = bacc.Bacc("TRN2", target_bir_lowering=False, debug=False)

    # ---- DRAM parameters ----
    pose_d = nc.dram_tensor("pose", [B_LOC, 72], F32, kind="ExternalInput")
    beta_d = nc.dram_tensor("beta", [B_LOC, 10], F32, kind="ExternalInput")
    dirs_d = nc.dram_tensor("dirs", [KC, 3, NV], BF16, kind="ExternalInput")
    wt_d = nc.dram_tensor("wt", [NJ, NV], BF16, kind="ExternalInput")
    js2_d = nc.dram_tensor("js2", [11, 72], F32, kind="ExternalInput")
    ident_d = nc.dram_tensor("ident", [128, 128], F32, kind="ExternalInput")
    out_d = nc.dram_tensor("out", [B_LOC, 3, NV], BF16, kind="ExternalOutput")

    with tile.TileContext(nc) as tc:
        with (
            tc.tile_pool(name="const", bufs=1) as constp,
            tc.tile_pool(name="state", bufs=1) as statep,
            tc.tile_pool(name="scr", bufs=1) as scrp,
        ):
            # ---- const loads ----
            ident = constp.tile([128, 128], F32)
            nc.sync.dma_start(ident[:, :], ident_d.ap())
            wt_sb = constp.tile([NJ, NV], BF16)
            nc.sync.dma_start(wt_sb[:, :], wt_d.ap())
            js2_sb = constp.tile([11, 72], F32)
            nc.sync.dma_start(js2_sb[:, :], js2_d.ap())
            pose_sb = statep.tile([B_LOC, 72], F32)
            nc.sync.dma_start(pose_sb[:, :], pose_d.ap())

            # ---- Rodrigues (fp32) ----
            V = nc.vector
            S = nc.scalar
            sq = scrp.tile([B_LOC, 72], F32, tag="sq")
            V.tensor_mul(sq[:, :], pose_sb[:, :], pose_sb[:, :])
            sq3 = sq[:, :].rearrange("p (j c) -> p c j", c=3)
            th2 = scrp.tile([B_LOC, NJ], F32, tag="th2")
            V.tensor_add(th2[:, :], sq3[:, 0, :], sq3[:, 1, :])
            V.tensor_add(th2[:, :], th2[:, :], sq3[:, 2, :])
            cbias = constp.tile([128, 2], F32)
            V.memset(cbias[:, 0:1], 1e-8)
            V.memset(cbias[:, 1:2], float(np.pi / 2))
            theta = scrp.tile([B_LOC, NJ], F32, tag="theta")
            S.activation(theta[:, :], th2[:, :], mybir.ActivationFunctionType.Sqrt,
                         bias=cbias[0:B_LOC, 0:1])
            invt = scrp.tile([B_LOC, NJ], F32, tag="invt")
            V.reciprocal(invt[:, :], theta[:, :])
            sh = scrp.tile([B_LOC, NJ], F32, tag="sh")
            S.activation(sh[:, :], theta[:, :], mybir.ActivationFunctionType.Sin, scale=0.5)
            chh = scrp.tile([B_LOC, NJ], F32, tag="chh")
            S.activation(chh[:, :], theta[:, :], mybir.ActivationFunctionType.Sin,
                         scale=0.5, bias=cbias[0:B_LOC, 1:2])
            s_t = scrp.tile([B_LOC, NJ], F32, tag="s_t")
            V.scalar_tensor_tensor(s_t[:, :], sh[:, :], 2.0, chh[:, :], AluOpType.mult, AluOpType.mult)
            shsq = scrp.tile([B_LOC, NJ], F32, tag="shsq")
            V.tensor_mul(shsq[:, :], sh[:, :], sh[:, :])
            c_t = scrp.tile([B_LOC, NJ], F32, tag="c_t")
            V.tensor_scalar(c_t[:, :], shsq[:, :], -2.0, 1.0, AluOpType.mult, AluOpType.add)
            omc = scrp.tile([B_LOC, NJ], F32, tag="omc")
            V.tensor_scalar_mul(omc[:, :], shsq[:, :], 2.0)
            ax = scrp.tile([B_LOC, 72], F32, tag="ax")
            ax3 = ax[:, :].rearrange("p (j c) -> p c j", c=3)
            p3 = pose_sb[:, :].rearrange("p (j c) -> p c j", c=3)
            for ci in range(3):
                V.tensor_mul(ax3[:, ci, :], p3[:, ci, :], invt[:, :])
            prods = {}
            for name, (a, b_) in {
                "xx": (0, 0), "yy": (1, 1), "zz": (2, 2),
                "xy": (0, 1), "xz": (0, 2), "yz": (1, 2),
            }.items():
                t = scrp.tile([B_LOC, NJ], F32, tag="prod_" + name)
                V.tensor_mul(t[:, :], ax3[:, a, :], ax3[:, b_, :])
                V.tensor_mul(t[:, :], t[:, :], omc[:, :])
                prods[name] = t
            for name, a in {"sx": 0, "sy": 1, "sz": 2}.items():
                t = scrp.tile([B_LOC, NJ], F32, tag="prod_" + name)
                V.tensor_mul(t[:, :], s_t[:, :], ax3[:, a, :])
                prods[name] = t
            r9 = statep.tile([B_LOC, NJ * 9], F32)
            r9e = r9[:, :].rearrange("p (j e) -> p e j", e=9)
            ENTRIES = [
                ("add", "c", "xx"), ("sub", "xy", "sz"), ("add", "xz", "sy"),
                ("add", "xy", "sz"), ("add", "c", "yy"), ("sub", "yz", "sx"),
                ("sub", "xz", "sy"), ("add", "yz", "sx"), ("add", "c", "zz"),
            ]
            for e, (op, a, b_) in enumerate(ENTRIES):
                ta = c_t if a == "c" else prods[a]
                fn = V.tensor_add if op == "add" else V.tensor_sub
                fn(r9e[:, e, :], ta[:, :], prods[b_][:, :])

            # ---- coeff & transposes ----
            coeff = statep.tile([B_LOC, KC], F32)
            nc.sync.dma_start(coeff[:, 0:10], beta_d.ap())
            V.tensor_copy(coeff[:, 10:217], r9[:, 9:216])
            lr9 = coeff[:, 10:217].rearrange("p (j e) -> p e j", e=9)
            for e in (0, 4, 8):
                V.tensor_scalar_add(lr9[:, e, :], lr9[:, e, :], -1.0)
            V.memset(coeff[:, 217:218], 1.0)

            with tc.tile_pool(name="psA", bufs=2, space="PSUM") as psA:
                pt1 = psA.tile([128, 128], F32, tag="tp")
                nc.tensor.transpose(pt1[:, :], coeff[:, 0:128], ident[:, :])
                coeffT_a = statep.tile([128, B_LOC], BF16)
                V.tensor_copy(coeffT_a[:, :], pt1[:, :])
                # fp32 betaT (rows 0..9 of pt1) + ones row for the J matmul
                betaT1 = statep.tile([11, B_LOC], F32)
                V.memset(betaT1[:, :], 1.0)
                V.tensor_copy(betaT1[0:10, :], pt1[0:10, :])
                pt2 = psA.tile([128, 128], F32, tag="tp")
                nc.tensor.transpose(pt2[0:90, :], coeff[:, 128:218], ident[:, :])
                coeffT_b = statep.tile([90, B_LOC], BF16)
                V.tensor_copy(coeffT_b[:, :], pt2[0:90, :])

                # ---- J = [beta | 1] @ JS2 (host-precomputed regressor) ----
                pj = psA.tile([B_LOC, 72], F32, tag="pj")
                nc.tensor.matmul(pj[:, :], betaT1[:, :], js2_sb[:, :],
                                 start=True, stop=True)
                j_sb = statep.tile([B_LOC, 72], F32)
                V.tensor_copy(j_sb[:, :], pj[:, :])

            # ---- J_rel ----
            jrel = statep.tile([B_LOC, 72], F32)
            jv = j_sb[:, :].rearrange("p (j c) -> p j c", c=3)
            jrv = jrel[:, :].rearrange("p (j c) -> p j c", c=3)
            V.tensor_copy(jrel[:, 0:3], j_sb[:, 0:3])
            V.tensor_sub(jrv[:, 1:4], jv[:, 1:4], jv[:, 0:1].broadcast_to([B_LOC, 3, 3]))
            V.tensor_sub(jrv[:, 4:12], jv[:, 4:12], jv[:, 1:9])
            V.tensor_sub(jrv[:, 12:15], jv[:, 12:15], jv[:, 9:10].broadcast_to([B_LOC, 3, 3]))
            V.tensor_sub(jrv[:, 15:18], jv[:, 15:18], jv[:, 12:15])
            V.tensor_sub(jrv[:, 18:24], jv[:, 18:24], jv[:, 16:22])

            # ---- local transforms Gl [128, 24*12] (3x4 row-major [R|t]) ----
            gl = statep.tile([B_LOC, NJ * 12], F32)
            gl4 = gl[:, :].rearrange("p (j m n) -> p j m n", m=3, n=4)
            r94 = r9[:, :].rearrange("p (j m n) -> p j m n", m=3, n=3)
            V.tensor_copy(gl4[:, :, :, 0:3], r94[:, :, :, :])
            V.tensor_copy(gl4[:, :, :, 3:4], jrv[:, :, :].unsqueeze(3))

            # ---- forward kinematics ----
            gw = statep.tile([B_LOC, NJ * 12], F32)
            gw4 = gw[:, :].rearrange("p (j m n) -> p j m n", m=3, n=4)
            V.tensor_copy(gw[:, 0:12], gl[:, 0:12])
            fktmp = scrp.tile([B_LOC, 3 * 12], F32, tag="fktmp")
            for (c0, ncld, p0, bc) in FK_GROUPS:
                child = gw4[:, c0:c0 + ncld]
                loc = gl4[:, c0:c0 + ncld]
                par = gw4[:, p0:p0 + (1 if bc else ncld)]
                tmpv = fktmp[:, 0:ncld * 12].rearrange("p (j m n) -> p j m n", m=3, n=4)
                shp = [B_LOC, ncld, 3, 4]
                for k in range(3):
                    in0 = loc[:, :, k:k + 1, :].broadcast_to(shp)
                    pk = par[:, 0:1, :, k:k + 1] if bc else par[:, :, :, k:k + 1]
                    in1 = pk.broadcast_to(shp)
                    if k == 0:
                        V.tensor_mul(child[:, :, :, :], in0, in1)
                    else:
                        V.tensor_mul(tmpv, in0, in1)
                        V.tensor_add(child[:, :, :, :], child[:, :, :, :], tmpv)
                ptr = par[:, 0:1, :, 3:4] if bc else par[:, :, :, 3:4]
                V.tensor_add(child[:, :, :, 3:4], child[:, :, :, 3:4],
                             ptr.broadcast_to([B_LOC, ncld, 3, 1]))

            # ---- rest-pose correction: t_j -= R_j^w @ J_j ----
            ct = scrp.tile([B_LOC, 72], F32, tag="ct")
            ct2 = scrp.tile([B_LOC, 72], F32, tag="ct2")
            ctv = ct[:, :].rearrange("p (j m) -> p j m", m=3).unsqueeze(3)
            ct2v = ct2[:, :].rearrange("p (j m) -> p j m", m=3).unsqueeze(3)
            for k in range(3):
                jk = jv[:, :, k:k + 1].unsqueeze(2).broadcast_to([B_LOC, NJ, 3, 1])
                if k == 0:
                    V.tensor_mul(ctv, gw4[:, :, :, k:k + 1], jk)
                else:
                    V.tensor_mul(ct2v, gw4[:, :, :, k:k + 1], jk)
                    V.tensor_add(ctv, ctv, ct2v)
            V.tensor_sub(gw4[:, :, :, 3:4], gw4[:, :, :, 3:4], ctv)

            # ---- gat via 12 transposes: [24, 12*128] bf16 ----
            gat = statep.tile([NJ, 12 * B_LOC], BF16)
            gwe = gw[:, :].rearrange("p (j e) -> p e j", e=12)
            with tc.tile_pool(name="psT", bufs=3, space="PSUM") as psT:
                for e in range(12):
                    pgt = psT.tile([NJ, B_LOC], F32, tag="gt")
                    nc.tensor.transpose(pgt[:, :], gwe[:, e, :], ident[:, :])
                    V.tensor_copy(gat[:, e * B_LOC:(e + 1) * B_LOC], pgt[:, :])

            # ---- main vertex-chunk loop ----
            _main_loop(nc, tc, coeffT_a, coeffT_b, gat, wt_sb, dirs_d, out_d)

    nc.compile()
    _CACHE[key] = nc
    return nc


def _main_loop(nc, tc, coeffT_a, coeffT_b, gat, wt_sb, dirs_d, out_d):
    V = nc.vector
    S = nc.scalar
    G = V
    ch = CH
    n_chunks = (NV + ch - 1) // ch
    with (
        tc.tile_pool(name="loop", bufs=3) as loopp,
        tc.tile_pool(name="tsb", bufs=2) as tsbp,
        tc.tile_pool(name="psVP", bufs=1, space="PSUM") as psVP,   # 3 banks
        tc.tile_pool(name="psTT", bufs=2, space="PSUM") as psTT,   # 2x2 banks
    ):
        for ci in range(n_chunks):
            v0 = ci * ch
            sz = min(ch, NV - v0)
            # dirs chunk: [K, 3, sz] bf16, two K-halves
            da = loopp.tile([128, 3, ch], BF16, tag="da")
            nc.sync.dma_start(da[:, :, 0:sz], dirs_d.ap()[0:128, :, v0:v0 + sz])
            db = loopp.tile([90, 3, ch], BF16, tag="db")
            nc.sync.dma_start(db[:, :, 0:sz], dirs_d.ap()[128:KC, :, v0:v0 + sz])

            # vp = coeff @ dirs  (3 c-planes x 2 K-halves)
            pvp = psVP.tile([128, 3 * ch], F32, tag="pvp")
            for c in range(3):
                nc.tensor.matmul(pvp[:, c * ch:c * ch + sz], coeffT_a[:, :],
                                 da[:, c, 0:sz], start=True, stop=False)
                nc.tensor.matmul(pvp[:, c * ch:c * ch + sz], coeffT_b[:, :],
                                 db[:, c, 0:sz], start=False, stop=True)
            # vp copy on ScalarE
            vp_sb = loopp.tile([B_LOC, 3, ch], BF16, tag="vp")
            if sz == ch:
                S.copy(vp_sb[:, :, :], pvp[:, :].rearrange("p (c v) -> p c v", v=ch))
            else:
                for c in range(3):
                    S.copy(vp_sb[:, c, 0:sz], pvp[:, c * ch:c * ch + sz])

            # T planes: 6 groups of 2, rotate psTT; groups 0-4 copied by
            # ScalarE, group 5 by DVE
            t_sb = tsbp.tile([B_LOC, 12, ch], BF16, tag="tsb")
            for g in range(6):
                ptt = psTT.tile([B_LOC, 2 * ch], F32, tag="ptt")
                for h in range(2):
                    e = g * 2 + h
                    nc.tensor.matmul(ptt[:, h * ch:h * ch + sz],
                                     gat[:, e * B_LOC:(e + 1) * B_LOC],
                                     wt_sb[:, v0:v0 + sz], start=True, stop=True)
                eng = S if g < 5 else V
                cp = eng.copy if eng is S else eng.tensor_copy
                if sz == ch:
                    cp(t_sb[:, g * 2:g * 2 + 2, :],
                       ptt[:, :].rearrange("p (h v) -> p h v", v=ch))
                else:
                    for h in range(2):
                        cp(t_sb[:, g * 2 + h, 0:sz], ptt[:, h * ch:h * ch + sz])

            # combine: out_m = sum_c T_{m*4+c} * vp_c + T_{m*4+3}
            pmul = loopp.tile([B_LOC, 3, 3, ch], BF16, tag="pmul")  # [p, c, m, v]
            for c in range(3):
                for m in range(3):
                    V.tensor_mul(pmul[:, c, m, 0:sz],
                                 t_sb[:, m * 4 + c, 0:sz], vp_sb[:, c, 0:sz])
            q = loopp.tile([B_LOC, 3, ch], BF16, tag="q")
            out_sb = loopp.tile([B_LOC, 3, ch], BF16, tag="outsb")
            t3 = t_sb[:, :, :].rearrange("p (m n) v -> p m n v", n=4)[:, :, 3]
            if sz == ch:
                G.tensor_add(q[:, :, :], pmul[:, 0], pmul[:, 1])
                G.tensor_add(q[:, :, :], q[:, :, :], pmul[:, 2])
                G.tensor_add(out_sb[:, :, :], q[:, :, :], t3)
            else:
                G.tensor_add(q[:, :, 0:sz], pmul[:, 0, :, 0:sz], pmul[:, 1, :, 0:sz])
                G.tensor_add(q[:, :, 0:sz], q[:, :, 0:sz], pmul[:, 2, :, 0:sz])
                G.tensor_add(out_sb[:, :, 0:sz], q[:, :, 0:sz], t3[:, :, 0:sz])
            nc.sync.dma_start(out_d.ap()[:, :, v0:v0 + sz], out_sb[:, :, 0:sz])


def _host_prep(inputs):
    shapedirs = np.asarray(inputs["shapedirs"], np.float32)    # [V,3,10]
    posedirs = np.asarray(inputs["posedirs"], np.float32)      # [V,3,207]
    v_template = np.asarray(inputs["v_template"], np.float32)  # [V,3]
    Jreg = np.asarray(inputs["J_regressor"], np.float32)       # [24,V]
    weights = np.asarray(inputs["weights"], np.float32)        # [V,24]

    dirs = np.empty((KC, 3, NV), np.float32)
    dirs[0:10] = shapedirs.transpose(2, 1, 0)
    dirs[10:217] = posedirs.transpose(2, 1, 0)
    dirs[217] = v_template.T
    # JS2: row s (s<10) = Jreg @ shapedirs[:,:,s] flattened [24*3]; row 10 = Jreg @ template
    js2 = np.empty((11, 72), np.float32)
    js2[0:10] = np.einsum('jv,vcs->sjc', Jreg, shapedirs).reshape(10, 72)
    js2[10] = (Jreg @ v_template).reshape(72)
    rep = {
        "dirs": np.ascontiguousarray(dirs.astype(NP_BF16)),
        "wt": np.ascontiguousarray(weights.T.astype(NP_BF16)),
        "js2": js2,
        "ident": np.eye(128, dtype=np.float32),
    }
    return rep


def kernel(pose, beta, shapedirs, posedirs, v_template, J_regressor, weights):
    cfg = CFG
    nc = build_program(cfg)
    rep = _host_prep(dict(shapedirs=shapedirs, posedirs=posedirs, v_template=v_template,
                          J_regressor=J_regressor, weights=weights))
    pose = np.asarray(pose, np.float32)
    beta = np.asarray(beta, np.float32)
    in_maps = []
    for i in range(N_CORES):
        m = dict(rep)
        m["pose"] = np.ascontiguousarray(pose[i * B_LOC:(i + 1) * B_LOC])
        m["beta"] = np.ascontiguousarray(beta[i * B_LOC:(i + 1) * B_LOC])
        in_maps.append(m)
    res = run_bass_kernel_spmd(nc, in_maps, core_ids=list(range(N_CORES)),
                               trace=cfg.get("trace", False))
    kernel.last_results = res
    out = np.concatenate([np.asarray(res.results[i]["out"]).astype(np.float32)
                          for i in range(N_CORES)], axis=0)
    return np.ascontiguousarray(out.transpose(0, 2, 1))


# revision 8
# speedup vs baseline: 1.4155x; 1.1955x over previous
"""SMPL (shape blend + pose blend + LBS skinning) Bass kernel for 8 TRN2 NeuronCores.

Data-parallel over batch: B=1024 -> 128 per core. All SMPL buffers replicated.

v2: bf16 main loop (fp32 matmuls are 4 cyc/row on TensorE; bf16 is 1),
host-precomputed joint regressor (J = [beta|1] @ JS2), ScalarE handles
PSUM->SBUF copies, DVE combine runs in bf16 2x mode.

Per-core pipeline:
  pose -> Rodrigues (fp32, ScalarE Sin half-angle + DVE) -> R [128,24*9]
  beta,lrotmin -> coeff [128,218] -> TensorE transpose -> coeffT (bf16 lhsT)
  J = [beta|1] @ JS2 (host-precomputed [11,72] regressor contraction)
  FK over the kinematic tree: 9 level-groups of broadcast-strided DVE ops
  G -> 12 TensorE transposes -> gat [24, 12*128] bf16 (lhsT of skinning matmul)
  main loop over vertex chunks (ch=512):
      vp   = coeffT.T @ dirs_chunk            (TensorE bf16, K=218)
      T_e  = gat_e.T @ wt_chunk               (TensorE bf16, K=24, 12 planes)
      out_m = sum_c T_mc*vp_c + T_m3          (DVE bf16 2x; ScalarE copies)
Output per core: [128, 3, 6890] bf16 plane-major; host reassembles [1024, 6890, 3].
"""

import sys
import numpy as np
import ml_dtypes

for _p in ("/opt/trn_rl_repo",):
    if _p not in sys.path:
        sys.path.append(_p)

import concourse.bass as bass
import concourse.tile as tile
import concourse.mybir as mybir
from concourse import bacc
from concourse.bass_utils import run_bass_kernel_spmd
from concourse.alu_op_type import AluOpType

F32 = mybir.dt.float32
BF16 = mybir.dt.bfloat16
NP_BF16 = ml_dtypes.bfloat16

N_CORES = 8
B = 1024
B_LOC = B // N_CORES  # 128
NV = 6890
NJ = 24
NPD = 207         # pose blend coeffs
KC = 218          # 10 beta + 207 lrotmin + 1 const
CH = 512          # vertex chunk

# FK level groups: (child_start, n_children, parent_start, parent_broadcast)
FK_GROUPS = [
    (1, 3, 0, True),
    (4, 3, 1, False),
    (7, 3, 4, False),
    (10, 3, 7, False),
    (13, 2, 9, True),
    (15, 3, 12, False),
    (18, 2, 16, False),
    (20, 2, 18, False),
    (22, 2, 20, False),
]

CFG = {
    "trace": False,
    "debug": False,
}

_CACHE = {}


def build_program(cfg):
    key = ("bf16", CH)
    if key in _CACHE:
        return _CACHE[key]

    nc = bacc.Bacc("TRN2", target_bir_lowering=False, debug=False)

    # ---- DRAM parameters ----
    pose_d = nc.dram_tensor("pose", [B_LOC, 72], F32, kind="ExternalInput")
    beta_d = nc.dram_tensor("beta", [B_LOC, 10], F32, kind="ExternalInput")
    dirs_d = nc.dram_tensor("dirs", [KC, 3, NV], BF16, kind="ExternalInput")
    wt_d = nc.dram_tensor("wt", [NJ, NV], BF16, kind="ExternalInput")
    js2_d = nc.dram_tensor("js2", [11, 72], F32, kind="ExternalInput")
    ident_d = nc.dram_tensor("ident", [128, 128], F32, kind="ExternalInput")
    out_d = nc.dram_tensor("out", [B_LOC, 3, NV], BF16, kind="ExternalOutput")

    with tile.TileContext(nc) as tc:
        with (
            tc.tile_pool(name="const", bufs=1) as constp,
            tc.tile_pool(name="state", bufs=1) as statep,
            tc.tile_pool(name="scr", bufs=1) as scrp,
        ):
            # ---- const loads ----
            ident = constp.tile([128, 128], F32)
            nc.sync.dma_start(ident[:, :], ident_d.ap())
            wt_sb = constp.tile([NJ, NV], BF16)
            nc.sync.dma_start(wt_sb[:, :], wt_d.ap())
            js2_sb = constp.tile([11, 72], F32)
            nc.sync.dma_start(js2_sb[:, :], js2_d.ap())
            pose_sb = statep.tile([B_LOC, 72], F32)
            nc.sync.dma_start(pose_sb[:, :], pose_d.ap())

            # ---- Rodrigues (fp32) ----
            V = nc.vector
            S = nc.scalar
            sq = scrp.tile([B_LOC, 72], F32, tag="sq")
            V.tensor_mul(sq[:, :], pose_sb[:, :], pose_sb[:, :])
            sq3 = sq[:, :].rearrange("p (j c) -> p c j", c=3)
            th2 = scrp.tile([B_LOC, NJ], F32, tag="th2")
            V.tensor_add(th2[:, :], sq3[:, 0, :], sq3[:, 1, :])
            V.tensor_add(th2[:, :], th2[:, :], sq3[:, 2, :])
            cbias = constp.tile([128, 2], F32)
            V.memset(cbias[:, 0:1], 1e-8)
            V.memset(cbias[:, 1:2], float(np.pi / 2))
            theta = scrp.tile([B_LOC, NJ], F32, tag="theta")
            S.activation(theta[:, :], th2[:, :], mybir.ActivationFunctionType.Sqrt,
                         bias=cbias[0:B_LOC, 0:1])
            invt = scrp.tile([B_LOC, NJ], F32, tag="invt")
            V.reciprocal(invt[:, :], theta[:, :])
            sh = scrp.tile([B_LOC, NJ], F32, tag="sh")
            S.activation(sh[:, :], theta[:, :], mybir.ActivationFunctionType.Sin, scale=0.5)
            chh = scrp.tile([B_LOC, NJ], F32, tag="chh")
            S.activation(chh[:, :], theta[:, :], mybir.ActivationFunctionType.Sin,
                         scale=0.5, bias=cbias[0:B_LOC, 1:2])
            s_t = scrp.tile([B_LOC, NJ], F32, tag="s_t")
            V.scalar_tensor_tensor(s_t[:, :], sh[:, :], 2.0, chh[:, :], AluOpType.mult, AluOpType.mult)
            shsq = scrp.tile([B_LOC, NJ], F32, tag="shsq")
            V.tensor_mul(shsq[:, :], sh[:, :], sh[:, :])
            c_t = scrp.tile([B_LOC, NJ], F32, tag="c_t")
            V.tensor_scalar(c_t[:, :], shsq[:, :], -2.0, 1.0, AluOpType.mult, AluOpType.add)
            omc = scrp.tile([B_LOC, NJ], F32, tag="omc")
            V.tensor_scalar_mul(omc[:, :], shsq[:, :], 2.0)
            ax = scrp.tile([B_LOC, 72], F32, tag="ax")
            ax3 = ax[:, :].rearrange("p (j c) -> p c j", c=3)
            p3 = pose_sb[:, :].rearrange("p (j c) -> p c j", c=3)
            for ci in range(3):
                V.tensor_mul(ax3[:, ci, :], p3[:, ci, :], invt[:, :])
            prods = {}
            for name, (a, b_) in {
                "xx": (0, 0), "yy": (1, 1), "zz": (2, 2),
                "xy": (0, 1), "xz": (0, 2), "yz": (1, 2),
            }.items():
                t = scrp.tile([B_LOC, NJ], F32, tag="prod_" + name)
                V.tensor_mul(t[:, :], ax3[:, a, :], ax3[:, b_, :])
                V.tensor_mul(t[:, :], t[:, :], omc[:, :])
                prods[name] = t
            for name, a in {"sx": 0, "sy": 1, "sz": 2}.items():
                t = scrp.tile([B_LOC, NJ], F32, tag="prod_" + name)
                V.tensor_mul(t[:, :], s_t[:, :], ax3[:, a, :])
                prods[name] = t
            r9 = statep.tile([B_LOC, NJ * 9], F32)
            r9e = r9[:, :].rearrange("p (j e) -> p e j", e=9)
            ENTRIES = [
                ("add", "c", "xx"), ("sub", "xy", "sz"), ("add", "xz", "sy"),
                ("add", "xy", "sz"), ("add", "c", "yy"), ("sub", "yz", "sx"),
                ("sub", "xz", "sy"), ("add", "yz", "sx"), ("add", "c", "zz"),
            ]
            for e, (op, a, b_) in enumerate(ENTRIES):
                ta = c_t if a == "c" else prods[a]
                fn = V.tensor_add if op == "add" else V.tensor_sub
                fn(r9e[:, e, :], ta[:, :], prods[b_][:, :])

            # ---- coeff & transposes ----
            coeff = statep.tile([B_LOC, KC], F32)
            nc.sync.dma_start(coeff[:, 0:10], beta_d.ap())
            V.tensor_copy(coeff[:, 10:217], r9[:, 9:216])
            lr9 = coeff[:, 10:217].rearrange("p (j e) -> p e j", e=9)
            for e in (0, 4, 8):
                V.tensor_scalar_add(lr9[:, e, :], lr9[:, e, :], -1.0)
            V.memset(coeff[:, 217:218], 1.0)

            with tc.tile_pool(name="psA", bufs=2, space="PSUM") as psA:
                pt1 = psA.tile([128, 128], F32, tag="tp")
                nc.tensor.transpose(pt1[:, :], coeff[:, 0:128], ident[:, :])
                coeffT_a = statep.tile([128, B_LOC], BF16)
                V.tensor_copy(coeffT_a[:, :], pt1[:, :])
                # fp32 betaT (rows 0..9 of pt1) + ones row for the J matmul
                betaT1 = statep.tile([11, B_LOC], F32)
                V.memset(betaT1[:, :], 1.0)
                V.tensor_copy(betaT1[0:10, :], pt1[0:10, :])
                pt2 = psA.tile([128, 128], F32, tag="tp")
                nc.tensor.transpose(pt2[0:90, :], coeff[:, 128:218], ident[:, :])
                coeffT_b = statep.tile([90, B_LOC], BF16)
                V.tensor_copy(coeffT_b[:, :], pt2[0:90, :])

                # ---- J = [beta | 1] @ JS2 (host-precomputed regressor) ----
                pj = psA.tile([B_LOC, 72], F32, tag="pj")
                nc.tensor.matmul(pj[:, :], betaT1[:, :], js2_sb[:, :],
                                 start=True, stop=True)
                j_sb = statep.tile([B_LOC, 72], F32)
                V.tensor_copy(j_sb[:, :], pj[:, :])

            # ---- J_rel ----
            jrel = statep.tile([B_LOC, 72], F32)
            jv = j_sb[:, :].rearrange("p (j c) -> p j c", c=3)
            jrv = jrel[:, :].rearrange("p (j c) -> p j c", c=3)
            V.tensor_copy(jrel[:, 0:3], j_sb[:, 0:3])
            V.tensor_sub(jrv[:, 1:4], jv[:, 1:4], jv[:, 0:1].broadcast_to([B_LOC, 3, 3]))
            V.tensor_sub(jrv[:, 4:12], jv[:, 4:12], jv[:, 1:9])
            V.tensor_sub(jrv[:, 12:15], jv[:, 12:15], jv[:, 9:10].broadcast_to([B_LOC, 3, 3]))
            V.tensor_sub(jrv[:, 15:18], jv[:, 15:18], jv[:, 12:15])
            V.tensor_sub(jrv[:, 18:24], jv[:, 18:24], jv[:, 16:22])

            # ---- local transforms Gl [128, 24*12] (3x4 row-major [R|t]) ----
            gl = statep.tile([B_LOC, NJ * 12], F32)
            gl4 = gl[:, :].rearrange("p (j m n) -> p j m n", m=3, n=4)
            r94 = r9[:, :].rearrange("p (j m n) -> p j m n", m=3, n=3)
            V.tensor_copy(gl4[:, :, :, 0:3], r94[:, :, :, :])
            V.tensor_copy(gl4[:, :, :, 3:4], jrv[:, :, :].unsqueeze(3))

            # ---- forward kinematics ----
            gw = statep.tile([B_LOC, NJ * 12], F32)
            gw4 = gw[:, :].rearrange("p (j m n) -> p j m n", m=3, n=4)
            V.tensor_copy(gw[:, 0:12], gl[:, 0:12])
            fktmp = scrp.tile([B_LOC, 3 * 12], F32, tag="fktmp")
            for (c0, ncld, p0, bc) in FK_GROUPS:
                child = gw4[:, c0:c0 + ncld]
                loc = gl4[:, c0:c0 + ncld]
                par = gw4[:, p0:p0 + (1 if bc else ncld)]
                tmpv = fktmp[:, 0:ncld * 12].rearrange("p (j m n) -> p j m n", m=3, n=4)
                shp = [B_LOC, ncld, 3, 4]
                for k in range(3):
                    in0 = loc[:, :, k:k + 1, :].broadcast_to(shp)
                    pk = par[:, 0:1, :, k:k + 1] if bc else par[:, :, :, k:k + 1]
                    in1 = pk.broadcast_to(shp)
                    if k == 0:
                        V.tensor_mul(child[:, :, :, :], in0, in1)
                    else:
                        V.tensor_mul(tmpv, in0, in1)
                        V.tensor_add(child[:, :, :, :], child[:, :, :, :], tmpv)
                ptr = par[:, 0:1, :, 3:4] if bc else par[:, :, :, 3:4]
                V.tensor_add(child[:, :, :, 3:4], child[:, :, :, 3:4],
                             ptr.broadcast_to([B_LOC, ncld, 3, 1]))

            # ---- rest-pose correction: t_j -= R_j^w @ J_j ----
            ct = scrp.tile([B_LOC, 72], F32, tag="ct")
            ct2 = scrp.tile([B_LOC, 72], F32, tag="ct2")
            ctv = ct[:, :].rearrange("p (j m) -> p j m", m=3).unsqueeze(3)
            ct2v = ct2[:, :].rearrange("p (j m) -> p j m", m=3).unsqueeze(3)
            for k in range(3):
                jk = jv[:, :, k:k + 1].unsqueeze(2).broadcast_to([B_LOC, NJ, 3, 1])
                if k == 0:
                    V.tensor_mul(ctv, gw4[:, :, :, k:k + 1], jk)
                else:
                    V.tensor_mul(ct2v, gw4[:, :, :, k:k + 1], jk)
                    V.tensor_add(ctv, ctv, ct2v)
            V.tensor_sub(gw4[:, :, :, 3:4], gw4[:, :, :, 3:4], ctv)

            # ---- gat via 12 transposes: [24, 12*128] bf16 ----
            gat = statep.tile([NJ, 12 * B_LOC], BF16)
            gwe = gw[:, :].rearrange("p (j e) -> p e j", e=12)
            with tc.tile_pool(name="psT", bufs=3, space="PSUM") as psT:
                for e in range(12):
                    pgt = psT.tile([NJ, B_LOC], F32, tag="gt")
                    nc.tensor.transpose(pgt[:, :], gwe[:, e, :], ident[:, :])
                    V.tensor_copy(gat[:, e * B_LOC:(e + 1) * B_LOC], pgt[:, :])

            # ---- main vertex-chunk loop ----
            _main_loop(nc, tc, coeffT_a, coeffT_b, gat, wt_sb, dirs_d, out_d)

    nc.compile()
    _CACHE[key] = nc
    return nc


def _main_loop(nc, tc, coeffT_a, coeffT_b, gat, wt_sb, dirs_d, out_d):
    V = nc.vector
    S = nc.scalar
    G = V
    ch = CH
    n_chunks = (NV + ch - 1) // ch
    with (
        tc.tile_pool(name="loop", bufs=3) as loopp,
        tc.tile_pool(name="tsb", bufs=2) as tsbp,
        tc.tile_pool(name="psVP", bufs=1, space="PSUM") as psVP,   # 3 banks
        tc.tile_pool(name="psTT", bufs=2, space="PSUM") as psTT,   # 2x2 banks
    ):
        for ci in range(n_chunks):
            v0 = ci * ch
            sz = min(ch, NV - v0)
            # dirs chunk: [K, 3, sz] bf16, two K-halves
            da = loopp.tile([128, 3, ch], BF16, tag="da")
            nc.sync.dma_start(da[:, :, 0:sz], dirs_d.ap()[0:128, :, v0:v0 + sz])
            db = loopp.tile([90, 3, ch], BF16, tag="db")
            nc.sync.dma_start(db[:, :, 0:sz], dirs_d.ap()[128:KC, :, v0:v0 + sz])

            # vp = coeff @ dirs  (3 c-planes x 2 K-halves)
            pvp = psVP.tile([128, 3 * ch], F32, tag="pvp")
            for c in range(3):
                nc.tensor.matmul(pvp[:, c * ch:c * ch + sz], coeffT_a[:, :],
                                 da[:, c, 0:sz], start=True, stop=False)
                nc.tensor.matmul(pvp[:, c * ch:c * ch + sz], coeffT_b[:, :],
                                 db[:, c, 0:sz], start=False, stop=True)
            # vp copy on ScalarE
            vp_sb = loopp.tile([B_LOC, 3, ch], BF16, tag="vp")
            if sz == ch:
                S.copy(vp_sb[:, :, :], pvp[:, :].rearrange("p (c v) -> p c v", v=ch))
            else:
                for c in range(3):
                    S.copy(vp_sb[:, c, 0:sz], pvp[:, c * ch:c * ch + sz])

            # T planes: 6 groups of 2, rotate psTT; groups 0-4 copied by
            # ScalarE, group 5 by DVE
            t_sb = tsbp.tile([B_LOC, 12, ch], BF16, tag="tsb")
            ptt5 = None
            for g in range(6):
                ptt = psTT.tile([B_LOC, 2 * ch], F32, tag="ptt")
                for h in range(2):
                    e = g * 2 + h
                    nc.tensor.matmul(ptt[:, h * ch:h * ch + sz],
                                     gat[:, e * B_LOC:(e + 1) * B_LOC],
                                     wt_sb[:, v0:v0 + sz], start=True, stop=True)
                if g < 5:
                    if sz == ch:
                        S.copy(t_sb[:, g * 2:g * 2 + 2, :],
                               ptt[:, :].rearrange("p (h v) -> p h v", v=ch))
                    else:
                        for h in range(2):
                            S.copy(t_sb[:, g * 2 + h, 0:sz], ptt[:, h * ch:h * ch + sz])
                else:
                    ptt5 = ptt

            # combine: out_m = sum_c T_{m*4+c} * vp_c + T_{m*4+3}
            # c in {0,1} muls first; group-5 copy on DVE after them so it
            # doesn't head-block the V queue; then c=2 muls.
            pmul = loopp.tile([B_LOC, 3, 3, ch], BF16, tag="pmul")  # [p, c, m, v]
            for c in range(2):
                for m in range(3):
                    V.tensor_mul(pmul[:, c, m, 0:sz],
                                 t_sb[:, m * 4 + c, 0:sz], vp_sb[:, c, 0:sz])
            if sz == ch:
                V.tensor_copy(t_sb[:, 10:12, :],
                              ptt5[:, :].rearrange("p (h v) -> p h v", v=ch))
            else:
                for h in range(2):
                    V.tensor_copy(t_sb[:, 10 + h, 0:sz], ptt5[:, h * ch:h * ch + sz])
            for m in range(3):
                V.tensor_mul(pmul[:, 2, m, 0:sz],
                             t_sb[:, m * 4 + 2, 0:sz], vp_sb[:, 2, 0:sz])
            q = loopp.tile([B_LOC, 3, ch], BF16, tag="q")
            out_sb = loopp.tile([B_LOC, 3, ch], BF16, tag="outsb")
            t3 = t_sb[:, :, :].rearrange("p (m n) v -> p m n v", n=4)[:, :, 3]
            if sz == ch:
                G.tensor_add(q[:, :, :], pmul[:, 0], pmul[:, 1])
                G.tensor_add(q[:, :, :], q[:, :, :], pmul[:, 2])
                G.tensor_add(out_sb[:, :, :], q[:, :, :], t3)
            else:
                G.tensor_add(q[:, :, 0:sz], pmul[:, 0, :, 0:sz], pmul[:, 1, :, 0:sz])
                G.tensor_add(q[:, :, 0:sz], q[:, :, 0:sz], pmul[:, 2, :, 0:sz])
                G.tensor_add(out_sb[:, :, 0:sz], q[:, :, 0:sz], t3[:, :, 0:sz])
            nc.sync.dma_start(out_d.ap()[:, :, v0:v0 + sz], out_sb[:, :, 0:sz])


def _host_prep(inputs):
    shapedirs = np.asarray(inputs["shapedirs"], np.float32)    # [V,3,10]
    posedirs = np.asarray(inputs["posedirs"], np.float32)      # [V,3,207]
    v_template = np.asarray(inputs["v_template"], np.float32)  # [V,3]
    Jreg = np.asarray(inputs["J_regressor"], np.float32)       # [24,V]
    weights = np.asarray(inputs["weights"], np.float32)        # [V,24]

    dirs = np.empty((KC, 3, NV), np.float32)
    dirs[0:10] = shapedirs.transpose(2, 1, 0)
    dirs[10:217] = posedirs.transpose(2, 1, 0)
    dirs[217] = v_template.T
    # JS2: row s (s<10) = Jreg @ shapedirs[:,:,s] flattened [24*3]; row 10 = Jreg @ template
    js2 = np.empty((11, 72), np.float32)
    js2[0:10] = np.einsum('jv,vcs->sjc', Jreg, shapedirs).reshape(10, 72)
    js2[10] = (Jreg @ v_template).reshape(72)
    rep = {
        "dirs": np.ascontiguousarray(dirs.astype(NP_BF16)),
        "wt": np.ascontiguousarray(weights.T.astype(NP_BF16)),
        "js2": js2,
        "ident": np.eye(128, dtype=np.float32),
    }
    return rep


def kernel(pose, beta, shapedirs, posedirs, v_template, J_regressor, weights):
    cfg = CFG
    nc = build_program(cfg)
    rep = _host_prep(dict(shapedirs=shapedirs, posedirs=posedirs, v_template=v_template,
                          J_regressor=J_regressor, weights=weights))
    pose = np.asarray(pose, np.float32)
    beta = np.asarray(beta, np.float32)
    in_maps = []
    for i in range(N_CORES):
        m = dict(rep)
        m["pose"] = np.ascontiguousarray(pose[i * B_LOC:(i + 1) * B_LOC])
        m["beta"] = np.ascontiguousarray(beta[i * B_LOC:(i + 1) * B_LOC])
        in_maps.append(m)
    res = run_bass_kernel_spmd(nc, in_maps, core_ids=list(range(N_CORES)),
                               trace=cfg.get("trace", False))
    kernel.last_results = res
    out = np.concatenate([np.asarray(res.results[i]["out"]).astype(np.float32)
                          for i in range(N_CORES)], axis=0)
    return np.ascontiguousarray(out.transpose(0, 2, 1))
